# revision 23
# baseline (speedup 1.0000x reference)
"""Trainium2 Bass kernel: location-sensitive (Tacotron2-style) attention.

  key  = memory @ W_memory.T                  [B, T, A]
  q    = query @ W_query.T                    [B, 1, A]
  loc  = (W_loc @ conv1d(att_cat, conv_w))^T  [B, T, A]
  e    = tanh(q + key + loc) @ v.T            [B, T]
  w    = softmax(e, axis=T)                   [B, T]
  ctx  = w @ memory                           [B, E]

Strategy: data-parallel over batch, 8 batches per NeuronCore, SPMD on 8
cores. All contractions run on the TensorEngine in bf16 with fp32 PSUM
accumulation. The host supplies `memory` in two layouts: [T, E] (context
contraction over T) and [E, T] (key contraction over E), both bf16, plus a
host-built im2col of the conv input. Energies are produced transposed
(T on partitions) so softmax normalization stays cheap; the exp-sum uses
the ScalarEngine's accum_out plus a ones-vector matmul for the partition
reduction. Softmax skips max-subtraction: energies = v . tanh(...) are
bounded by ||v||_1 < 1 for this problem family, so exp never overflows.
"""

import numpy as np
import ml_dtypes

import concourse.bass as bass
import concourse.tile as tile
from concourse import bacc, bass_isa, mybir
from concourse.bass_utils import run_bass_kernel_spmd

BF16 = ml_dtypes.bfloat16
N_CORES = 8
B, T, DQ, A, E, NF, KW = 64, 2048, 1024, 128, 512, 32, 31
PAD = (KW - 1) // 2
BL = B // N_CORES          # batches per core
CK = 62                    # im2col rows = 2 * KW
TJ = T // 512              # 4 T-chunks of 512 for energies
TC = T // 128              # 16 T-chunks of 128 for context
EC = E // 128              # 4 E-chunks of 128
QC = DQ // 128             # 8 DQ-chunks of 128

F32 = mybir.dt.float32
BF = mybir.dt.bfloat16
FP8 = mybir.dt.float8e4
FP8NP = ml_dtypes.float8_e4m3
ESCALE = 256.0
VSCALE = 64.0
AF = mybir.ActivationFunctionType

_cache = {}
LAST_RESULTS = None


def _build():
    nc = bacc.Bacc(
        "TRN2",
        target_bir_lowering=False,
        debug=False,
        enable_asserts=False,
        num_devices=N_CORES,
    )
    memT_d = nc.dram_tensor("memT", [BL, 128, EC, T], FP8, kind="ExternalInput").ap()
    memoA_d = nc.dram_tensor("memoA", [BL, 128, 3, E], BF, kind="ExternalInput").ap()
    memoB_d = nc.dram_tensor("memoB", [BL, 128, TC - 3, E], BF, kind="ExternalInput").ap()
    i2c_d = nc.dram_tensor("i2c", [BL, CK, T], BF, kind="ExternalInput").ap()
    wmT_d = nc.dram_tensor("wmT", [128, EC, 128], FP8, kind="ExternalInput").ap()
    wqT_d = nc.dram_tensor("wqT", [128, QC, 128], BF, kind="ExternalInput").ap()
    cw_d = nc.dram_tensor("cw", [NF, CK], BF, kind="ExternalInput").ap()
    wlT_d = nc.dram_tensor("wlT", [NF, A], BF, kind="ExternalInput").ap()
    vT_d = nc.dram_tensor("vT", [A, 1], BF, kind="ExternalInput").ap()
    qin_d = nc.dram_tensor("qin", [128, QC, BL], BF, kind="ExternalInput").ap()
    ident_d = nc.dram_tensor("ident", [128, 128], F32, kind="ExternalInput").ap()
    ctx_d = nc.dram_tensor("ctx", [BL, E], F32, kind="ExternalOutput").ap()
    wgt_d = nc.dram_tensor("wgt", [BL, T], F32, kind="ExternalOutput").ap()

    with tile.TileContext(nc) as tc:
        from contextlib import ExitStack

        with ExitStack() as ctx:
            consts = ctx.enter_context(tc.tile_pool(name="consts", bufs=1))
            memT_p = ctx.enter_context(tc.tile_pool(name="memT_p", bufs=5))
            memo_p = ctx.enter_context(tc.tile_pool(name="memo_p", bufs=4))
            i2c_p = ctx.enter_context(tc.tile_pool(name="i2c_p", bufs=3))
            th_p = ctx.enter_context(tc.tile_pool(name="th_p", bufs=4))
            sm_p = ctx.enter_context(tc.tile_pool(name="sm_p", bufs=3))
            ps_e = ctx.enter_context(tc.tile_pool(name="ps_e", bufs=1, space="PSUM"))
            ps_et = ctx.enter_context(tc.tile_pool(name="ps_et", bufs=2, space="PSUM"))
            ps_tiny = ctx.enter_context(
                tc.tile_pool(name="ps_tiny", bufs=2, space="PSUM")
            )

            # ---- constants ----
            wm = consts.tile([128, EC, 128], FP8, name="wm")
            nc.sync.dma_start(wm[:], wmT_d[:])
            wq = consts.tile([128, QC, 128], BF, name="wq")
            nc.sync.dma_start(wq[:], wqT_d[:])
            cw = consts.tile([NF, CK], BF, name="cw")
            nc.sync.dma_start(cw[:], cw_d[:])
            wlT = consts.tile([NF, A], BF, name="wlT")
            nc.sync.dma_start(wlT[:], wlT_d[:])
            vt = consts.tile([A, 1], BF, name="vt")
            nc.sync.dma_start(vt[:], vT_d[:])
            qin = consts.tile([128, QC, BL], BF, name="qin")
            nc.sync.dma_start(qin[:], qin_d[:])
            ident = consts.tile([128, 128], F32, name="ident")
            nc.gpsimd.dma_start(ident[:], ident_d[:])

            # PE warmup: dummy matmuls so the HAM clock-gate goes K=8/8 while
            # the first batch's DMAs land; deps only on a DVE memset
            warm_sb = consts.tile([128, 512], BF, name="warm_sb")
            nc.vector.memset(warm_sb[:], 0.5)
            warm_w = consts.tile([128, 128], BF, name="warm_w")
            nc.vector.memset(warm_w[:], 0.5)
            warm_ps = ps_tiny.tile([128, 512], F32, name="warm_ps", tag="tiny")
            for wi in range(14):
                nc.tensor.matmul(
                    warm_ps[:], warm_w[:], warm_sb[:],
                    start=(wi == 0), stop=(wi == 13),
                )

            # fused location weights: mt[j, a] = sum_f cw[f, j] * W_loc[a, f]
            mt_ps = ps_tiny.tile([CK, A], F32, name="mt_ps", tag="tiny")
            nc.tensor.matmul(mt_ps[:], cw[:], wlT[:], start=True, stop=True)
            mt = consts.tile([CK, A], BF, name="mt")
            nc.vector.tensor_copy(mt[:], mt_ps[:])

            # projected query for all local batches: qt[a, b]
            q_ps = ps_tiny.tile([A, BL], F32, name="q_ps", tag="tiny")
            for c in range(QC):
                nc.tensor.matmul(
                    q_ps[:], wq[:, c, :], qin[:, c, :],
                    start=(c == 0), stop=(c == QC - 1),
                )
            qt = consts.tile([A, BL], F32, name="qt")
            nc.vector.tensor_copy(qt[:], q_ps[:])

            def emit_ctx(st):
                # context: ctx[e] = sum_t exp[t] * mem[t, e], scaled by 1/sum.
                # Emitted one batch late so these PE matmuls fill the PE idle
                # window while the ScalarEngine runs the next batch's tanhs.
                bb, wT16, memt, rep_s = st
                ctx_ps = ps_tiny.tile([1, E], F32, name="ctx_ps", tag="tiny")
                for j2 in range(TC):
                    nc.tensor.matmul(
                        ctx_ps[:], wT16[:, j2 : j2 + 1], memt[:, j2, :],
                        start=(j2 == 0), stop=(j2 == TC - 1),
                    )
                ctxs = sm_p.tile([1, E], F32, name="ctxs")
                nc.scalar.mul(ctxs[:], ctx_ps[:], mul=rep_s[0:1, :])
                nc.gpsimd.dma_start(ctx_d[bb : bb + 1, :], ctxs[:])

            pending = None
            for b in range(BL):
                memTt = memT_p.tile([128, EC, T], FP8, name="memTt")
                nc.sync.dma_start(memTt[:], memT_d[b])
                i2c = i2c_p.tile([CK, T], BF, name="i2c")
                nc.sync.dma_start(i2c[:], i2c_d[b])
                memt = memo_p.tile([128, TC, E], BF, name="memt")
                nc.sync.dma_start(memt[:, 0:3, :], memoA_d[b])
                nc.scalar.dma_start(memt[:, 3:, :], memoB_d[b])

                # energies, transposed: et_ps[p, col] = e[t = col*128 + p]
                # stationary-outer ordering: one stationary feeds 4
                # back-to-back N=512 matmuls into 4 psum banks
                et_ps = ps_et.tile([128, TC], F32, name="et_ps")
                e_ps = ps_e.tile([128, TJ, 512], F32, name="e_ps")
                for c in range(0, EC, 2):
                    for j in range(TJ):
                        nc.tensor.matmul(
                            e_ps[:, j, :], wm[:, c : c + 2, :],
                            memTt[:, c : c + 2, bass.ts(j, 512)],
                            perf_mode=mybir.MatmulPerfMode.DoubleRow,
                            start=(c == 0), stop=False,
                        )
                for j in range(TJ):
                    nc.tensor.matmul(
                        e_ps[:, j, :], mt[:], i2c[:, bass.ts(j, 512)],
                        start=False, stop=True,
                    )

                if pending is not None:
                    emit_ctx(pending)

                for j in range(TJ):
                    th = th_p.tile([128, 512], BF, name="th")
                    nc.scalar.activation(
                        th[:], e_ps[:, j, :], AF.Tanh,
                        bias=qt[:, b : b + 1], scale=1.0 / ESCALE,
                    )
                    for j2 in range(4):
                        col = j * 4 + j2
                        nc.tensor.matmul(
                            et_ps[:, col : col + 1],
                            th[:, bass.ts(j2, 128)], vt[:],
                            start=True, stop=True,
                        )

                # softmax over T (T on partitions x 16 cols); no max-sub needed.
                # exp's accum_out gives per-partition sums; a GpSimd partition
                # all-reduce + DVE reciprocal yields 1/sum replicated on all
                # partitions with zero TensorEngine work.
                expT = sm_p.tile([128, TC], F32, name="expT")
                acc = sm_p.tile([128, 1], F32, name="acc")
                nc.scalar.activation(expT[:], et_ps[:], AF.Exp, accum_out=acc[:])
                sumr = sm_p.tile([128, 1], F32, name="sumr")
                nc.gpsimd.partition_all_reduce(
                    sumr[:], acc[:], channels=128, reduce_op=bass_isa.ReduceOp.add
                )
                rep_s = sm_p.tile([128, 1], F32, name="rep_s")
                nc.vector.reciprocal(rep_s[:], sumr[:])

                # weights output: wgt[b, t] = expT / sum, back in t-major layout
                wtr_ps = ps_tiny.tile([TC, 128], F32, name="wtr_ps", tag="tiny")
                nc.tensor.transpose(wtr_ps[:], expT[:], ident[:])
                wrow = sm_p.tile([TC, 128], F32, name="wrow")
                nc.vector.tensor_scalar_mul(wrow[:], wtr_ps[:], rep_s[0:TC, :])
                nc.gpsimd.dma_start(wgt_d[b].rearrange("(r t) -> r t", r=TC), wrow[:])

                wT16 = sm_p.tile([128, TC], BF, name="wT16")
                nc.vector.tensor_copy(wT16[:], expT[:])
                pending = (b, wT16, memt, rep_s)

            emit_ctx(pending)

    nc.compile()
    return nc


def _prep(query, memory, attention_weights_cat, W_query, W_memory, conv_w, W_loc, v):
    """Host-side layout prep + per-core sharding. memT/wmT are fp8e4m3 with
    weights pre-scaled by 256 (energies rescaled in the tanh activation)."""
    query = np.asarray(query, np.float32)
    memory = np.asarray(memory, np.float32)
    att = np.asarray(attention_weights_cat, np.float32)
    W_query = np.asarray(W_query, np.float32)
    W_memory = np.asarray(W_memory, np.float32)
    conv_w = np.asarray(conv_w, np.float32)
    W_loc = np.asarray(W_loc, np.float32)
    v = np.asarray(v, np.float32)

    # packed so each SBUF partition's payload is one contiguous DRAM chunk:
    # memT[b, p, c, t] = memory[b, t, c*128 + p];  memo[b, p, j, e] = memory[b, j*128 + p, e]
    memT = np.ascontiguousarray(
        memory.transpose(0, 2, 1).reshape(B, EC, 128, T).transpose(0, 2, 1, 3)
    ).astype(FP8NP)
    memo = memory.reshape(B, TC, 128, E).transpose(0, 2, 1, 3).astype(BF16)
    memoA = np.ascontiguousarray(memo[:, :, 0:3, :])
    memoB = np.ascontiguousarray(memo[:, :, 3:, :])
    x_pad = np.zeros((B, 2, T + 2 * PAD), np.float32)
    x_pad[:, :, PAD : PAD + T] = att
    i2c = (
        np.lib.stride_tricks.sliding_window_view(x_pad, T, axis=2)
        .reshape(B, CK, T)
        .astype(BF16)
    )
    wmT = np.ascontiguousarray(
        256.0 * W_memory.T.reshape(EC, 128, A).transpose(1, 0, 2)
    ).astype(FP8NP)
    wqT = np.ascontiguousarray(
        W_query.T.reshape(QC, 128, A).transpose(1, 0, 2)
    ).astype(BF16)
    cw = np.ascontiguousarray(conv_w.reshape(NF, CK)).astype(BF16)
    wlT = np.ascontiguousarray(256.0 * W_loc.T).astype(BF16)
    vT = np.ascontiguousarray(v.reshape(1, A).T).astype(BF16)
    qT = query.reshape(B, DQ).T.reshape(QC, 128, B).transpose(1, 0, 2).astype(BF16)  # [128, QC, B]
    ident = np.eye(128, dtype=np.float32)

    in_maps = []
    for m in range(N_CORES):
        s = slice(m * BL, (m + 1) * BL)
        in_maps.append(
            {
                "memT": np.ascontiguousarray(memT[s]),
                "memoA": memoA[s],
                "memoB": memoB[s],
                "i2c": np.ascontiguousarray(i2c[s]),
                "wmT": wmT,
                "wqT": wqT,
                "cw": cw,
                "wlT": wlT,
                "vT": vT,
                "qin": np.ascontiguousarray(qT[:, :, s]),
                "ident": ident,
            }
        )
    return in_maps


def kernel(query, memory, attention_weights_cat, W_query, W_memory, conv_w, W_loc, v):
    global LAST_RESULTS
    if "nc" not in _cache:
        _cache["nc"] = _build()
    nc = _cache["nc"]
    in_maps = _prep(
        query, memory, attention_weights_cat, W_query, W_memory, conv_w, W_loc, v
    )
    import os

    trace = bool(os.environ.get("BASS_TRACE"))
    res = run_bass_kernel_spmd(
        nc, in_maps, core_ids=list(range(N_CORES)), trace=trace
    )
    LAST_RESULTS = res
    ctx_full = np.concatenate([r["ctx"] for r in res.results], axis=0)
    wgt_full = np.concatenate([r["wgt"] for r in res.results], axis=0)
    return ctx_full, wgt_full


# revision 33
# speedup vs baseline: 1.1474x; 1.1474x over previous
"""Trainium2 Bass kernel: location-sensitive (Tacotron2-style) attention.

  key  = memory @ W_memory.T                  [B, T, A]
  q    = query @ W_query.T                    [B, 1, A]
  loc  = (W_loc @ conv1d(att_cat, conv_w))^T  [B, T, A]
  e    = tanh(q + key + loc) @ v.T            [B, T]
  w    = softmax(e, axis=T)                   [B, T]
  ctx  = w @ memory                           [B, E]

Strategy: data-parallel over batch, 8 batches per NeuronCore, SPMD on 8
cores. All contractions run on the TensorEngine in bf16 with fp32 PSUM
accumulation. The host supplies `memory` in two layouts: [T, E] (context
contraction over T) and [E, T] (key contraction over E), both bf16, plus a
host-built im2col of the conv input. Energies are produced transposed
(T on partitions) so softmax normalization stays cheap; the exp-sum uses
the ScalarEngine's accum_out plus a ones-vector matmul for the partition
reduction. Softmax skips max-subtraction: energies = v . tanh(...) are
bounded by ||v||_1 < 1 for this problem family, so exp never overflows.
"""

import numpy as np
import ml_dtypes

import concourse.bass as bass
import concourse.tile as tile
from concourse import bacc, bass_isa, mybir
from concourse.bass_utils import run_bass_kernel_spmd

BF16 = ml_dtypes.bfloat16
N_CORES = 8
B, T, DQ, A, E, NF, KW = 64, 2048, 1024, 128, 512, 32, 31
PAD = (KW - 1) // 2
BL = B // N_CORES          # batches per core
CK = 62                    # im2col rows = 2 * KW
TJ = T // 512              # 4 T-chunks of 512 for energies
TC = T // 128              # 16 T-chunks of 128 for context
EC = E // 128              # 4 E-chunks of 128
QC = DQ // 128             # 8 DQ-chunks of 128

F32 = mybir.dt.float32
BF = mybir.dt.bfloat16
FP8 = mybir.dt.float8e4
FP8NP = ml_dtypes.float8_e4m3
ESCALE = 256.0
VSCALE = 64.0
AF = mybir.ActivationFunctionType

_cache = {}
LAST_RESULTS = None


def _build():
    nc = bacc.Bacc(
        "TRN2",
        target_bir_lowering=False,
        debug=False,
        enable_asserts=False,
        num_devices=N_CORES,
    )
    memT_d = nc.dram_tensor("memT", [BL, 128, EC, T], FP8, kind="ExternalInput").ap()
    memoA_d = nc.dram_tensor("memoA", [BL, 128, 3, E], BF, kind="ExternalInput").ap()
    memoB_d = nc.dram_tensor("memoB", [BL, 128, TC - 3, E], BF, kind="ExternalInput").ap()
    i2c_d = nc.dram_tensor("i2c", [BL, CK, T], FP8, kind="ExternalInput").ap()
    wmT_d = nc.dram_tensor("wmT", [128, EC, 128], FP8, kind="ExternalInput").ap()
    wqT_d = nc.dram_tensor("wqT", [128, QC, 128], BF, kind="ExternalInput").ap()
    cw_d = nc.dram_tensor("cw", [NF, CK], BF, kind="ExternalInput").ap()
    wlT_d = nc.dram_tensor("wlT", [NF, A], BF, kind="ExternalInput").ap()
    vT_d = nc.dram_tensor("vT", [A, 1], BF, kind="ExternalInput").ap()
    qin_d = nc.dram_tensor("qin", [128, QC, BL], BF, kind="ExternalInput").ap()
    ident_d = nc.dram_tensor("ident", [128, 128], F32, kind="ExternalInput").ap()
    ctx_d = nc.dram_tensor("ctx", [BL, E], F32, kind="ExternalOutput").ap()
    wgt_d = nc.dram_tensor("wgt", [BL, T], F32, kind="ExternalOutput").ap()

    with tile.TileContext(nc) as tc:
        from contextlib import ExitStack

        with ExitStack() as ctx:
            consts = ctx.enter_context(tc.tile_pool(name="consts", bufs=1))
            memT_p = ctx.enter_context(tc.tile_pool(name="memT_p", bufs=5))
            memo_p = ctx.enter_context(tc.tile_pool(name="memo_p", bufs=4))
            i2c_p = ctx.enter_context(tc.tile_pool(name="i2c_p", bufs=3))
            th_p = ctx.enter_context(tc.tile_pool(name="th_p", bufs=4))
            sm_p = ctx.enter_context(tc.tile_pool(name="sm_p", bufs=3))
            ps_e = ctx.enter_context(tc.tile_pool(name="ps_e", bufs=1, space="PSUM"))
            ps_et = ctx.enter_context(tc.tile_pool(name="ps_et", bufs=2, space="PSUM"))
            ps_tiny = ctx.enter_context(
                tc.tile_pool(name="ps_tiny", bufs=2, space="PSUM")
            )

            # ---- constants ----
            wm = consts.tile([128, EC, 128], FP8, name="wm")
            nc.sync.dma_start(wm[:], wmT_d[:])
            wq = consts.tile([128, QC, 128], BF, name="wq")
            nc.sync.dma_start(wq[:], wqT_d[:])
            cw = consts.tile([NF, CK], BF, name="cw")
            nc.sync.dma_start(cw[:], cw_d[:])
            wlT = consts.tile([NF, A], BF, name="wlT")
            nc.sync.dma_start(wlT[:], wlT_d[:])
            vt = consts.tile([A, 1], BF, name="vt")
            nc.sync.dma_start(vt[:], vT_d[:])
            qin = consts.tile([128, QC, BL], BF, name="qin")
            nc.sync.dma_start(qin[:], qin_d[:])
            ident = consts.tile([128, 128], F32, name="ident")
            nc.gpsimd.dma_start(ident[:], ident_d[:])

            # PE warmup: dummy matmuls so the HAM clock-gate goes K=8/8 while
            # the first batch's DMAs land; deps only on a DVE memset
            warm_sb = consts.tile([128, 512], BF, name="warm_sb")
            nc.vector.memset(warm_sb[:], 0.5)
            warm_w = consts.tile([128, 128], BF, name="warm_w")
            nc.vector.memset(warm_w[:], 0.5)
            warm_ps = ps_tiny.tile([128, 512], F32, name="warm_ps", tag="tiny")
            for wi in range(14):
                nc.tensor.matmul(
                    warm_ps[:], warm_w[:], warm_sb[:],
                    start=(wi == 0), stop=(wi == 13),
                )

            # fused location weights: mt[j, a] = sum_f cw[f, j] * W_loc[a, f]
            mt_ps = ps_tiny.tile([CK, A], F32, name="mt_ps", tag="tiny")
            nc.tensor.matmul(mt_ps[:], cw[:], wlT[:], start=True, stop=True)
            mt = consts.tile([CK, A], FP8, name="mt")
            nc.vector.tensor_copy(mt[:], mt_ps[:])

            # projected query for all local batches: qt[a, b]
            q_ps = ps_tiny.tile([A, BL], F32, name="q_ps", tag="tiny")
            for c in range(QC):
                nc.tensor.matmul(
                    q_ps[:], wq[:, c, :], qin[:, c, :],
                    start=(c == 0), stop=(c == QC - 1),
                )
            qt = consts.tile([A, BL], F32, name="qt")
            nc.vector.tensor_copy(qt[:], q_ps[:])

            def emit_ctx(st):
                # context: ctx[e] = sum_t exp[t] * mem[t, e], scaled by 1/sum.
                # Emitted one batch late so these PE matmuls fill the PE idle
                # window while the ScalarEngine runs the next batch's tanhs.
                bb, wT16, memt, rep_s = st
                ctx_ps = ps_tiny.tile([1, E], F32, name="ctx_ps", tag="tiny")
                for j2 in range(TC):
                    nc.tensor.matmul(
                        ctx_ps[:], wT16[:, j2 : j2 + 1], memt[:, j2, :],
                        start=(j2 == 0), stop=(j2 == TC - 1),
                    )
                ctxs = sm_p.tile([1, E], F32, name="ctxs")
                nc.scalar.mul(ctxs[:], ctx_ps[:], mul=rep_s[0:1, :])
                nc.gpsimd.dma_start(ctx_d[bb : bb + 1, :], ctxs[:])

            pending = None
            for b in range(BL):
                i2c = i2c_p.tile([CK, T], FP8, name="i2c")
                nc.sync.dma_start(i2c[:], i2c_d[b])
                memTt = memT_p.tile([128, EC, T], FP8, name="memTt")
                nc.sync.dma_start(memTt[:], memT_d[b])
                memt = memo_p.tile([128, TC, E], BF, name="memt")
                nc.sync.dma_start(memt[:, 0:3, :], memoA_d[b])
                nc.scalar.dma_start(memt[:, 3:, :], memoB_d[b])

                # energies, transposed: et_ps[p, col] = e[t = col*128 + p]
                # stationary-outer ordering: one stationary feeds 4
                # back-to-back N=512 matmuls into 4 psum banks
                et_ps = ps_et.tile([128, TC], F32, name="et_ps")
                e_ps = ps_e.tile([128, TJ, 512], F32, name="e_ps", tag="e_ps")
                for c in range(0, EC, 2):
                    for j in range(TJ):
                        nc.tensor.matmul(
                            e_ps[:, j, :], wm[:, c : c + 2, :],
                            memTt[:, c : c + 2, bass.ts(j, 512)],
                            perf_mode=mybir.MatmulPerfMode.DoubleRow,
                            start=(c == 0), stop=False,
                        )
                for j in range(TJ):
                    nc.tensor.matmul(
                        e_ps[:, j, :], mt[:], i2c[:, bass.ts(j, 512)],
                        start=False, stop=True,
                    )

                if pending is not None:
                    emit_ctx(pending)

                for j in range(TJ):
                    th = th_p.tile([128, 512], BF, name="th")
                    nc.scalar.activation(
                        th[:], e_ps[:, j, :], AF.Tanh,
                        bias=qt[:, b : b + 1], scale=1.0 / ESCALE,
                    )
                    for j2 in range(4):
                        col = j * 4 + j2
                        nc.tensor.matmul(
                            et_ps[:, col : col + 1],
                            th[:, bass.ts(j2, 128)], vt[:],
                            start=True, stop=True,
                        )

                # softmax over T (T on partitions x 16 cols); no max-sub needed.
                # exp's accum_out gives per-partition sums; a GpSimd partition
                # all-reduce + DVE reciprocal yields 1/sum replicated on all
                # partitions with zero TensorEngine work.
                expT = sm_p.tile([128, TC], F32, name="expT")
                acc = sm_p.tile([128, 1], F32, name="acc")
                nc.scalar.activation(expT[:], et_ps[:], AF.Exp, accum_out=acc[:])
                sumr = sm_p.tile([128, 1], F32, name="sumr")
                nc.gpsimd.partition_all_reduce(
                    sumr[:], acc[:], channels=128, reduce_op=bass_isa.ReduceOp.add
                )
                rep_s = sm_p.tile([128, 1], F32, name="rep_s")
                nc.vector.reciprocal(rep_s[:], sumr[:])

                # weights output: wgt[b, t] = expT / sum, back in t-major layout
                wtr_ps = ps_tiny.tile([TC, 128], F32, name="wtr_ps", tag="tiny")
                nc.tensor.transpose(wtr_ps[:], expT[:], ident[:])
                wrow = sm_p.tile([TC, 128], F32, name="wrow")
                nc.vector.tensor_scalar_mul(wrow[:], wtr_ps[:], rep_s[0:TC, :])
                nc.gpsimd.dma_start(wgt_d[b].rearrange("(r t) -> r t", r=TC), wrow[:])

                wT16 = sm_p.tile([128, TC], BF, name="wT16")
                nc.vector.tensor_copy(wT16[:], expT[:])
                pending = (b, wT16, memt, rep_s)

            emit_ctx(pending)

    nc.compile()
    return nc


def _prep(query, memory, attention_weights_cat, W_query, W_memory, conv_w, W_loc, v):
    """Host-side layout prep + per-core sharding. memT/wmT are fp8e4m3 with
    weights pre-scaled by 256 (energies rescaled in the tanh activation)."""
    query = np.asarray(query, np.float32)
    memory = np.asarray(memory, np.float32)
    att = np.asarray(attention_weights_cat, np.float32)
    W_query = np.asarray(W_query, np.float32)
    W_memory = np.asarray(W_memory, np.float32)
    conv_w = np.asarray(conv_w, np.float32)
    W_loc = np.asarray(W_loc, np.float32)
    v = np.asarray(v, np.float32)

    # packed so each SBUF partition's payload is one contiguous DRAM chunk:
    # memT[b, p, c, t] = memory[b, t, c*128 + p];  memo[b, p, j, e] = memory[b, j*128 + p, e]
    memT = np.ascontiguousarray(
        memory.transpose(0, 2, 1).reshape(B, EC, 128, T).transpose(0, 2, 1, 3)
    ).astype(FP8NP)
    memo = memory.reshape(B, TC, 128, E).transpose(0, 2, 1, 3).astype(BF16)
    memoA = np.ascontiguousarray(memo[:, :, 0:3, :])
    memoB = np.ascontiguousarray(memo[:, :, 3:, :])
    x_pad = np.zeros((B, 2, T + 2 * PAD), np.float32)
    x_pad[:, :, PAD : PAD + T] = att
    i2c = (
        np.lib.stride_tricks.sliding_window_view(x_pad, T, axis=2)
        .reshape(B, CK, T)
        .astype(FP8NP)
    )
    wmT = np.ascontiguousarray(
        256.0 * W_memory.T.reshape(EC, 128, A).transpose(1, 0, 2)
    ).astype(FP8NP)
    wqT = np.ascontiguousarray(
        W_query.T.reshape(QC, 128, A).transpose(1, 0, 2)
    ).astype(BF16)
    cw = np.ascontiguousarray(conv_w.reshape(NF, CK)).astype(BF16)
    wlT = np.ascontiguousarray(256.0 * W_loc.T).astype(BF16)
    vT = np.ascontiguousarray(v.reshape(1, A).T).astype(BF16)
    qT = query.reshape(B, DQ).T.reshape(QC, 128, B).transpose(1, 0, 2).astype(BF16)  # [128, QC, B]
    ident = np.eye(128, dtype=np.float32)

    in_maps = []
    for m in range(N_CORES):
        s = slice(m * BL, (m + 1) * BL)
        in_maps.append(
            {
                "memT": np.ascontiguousarray(memT[s]),
                "memoA": memoA[s],
                "memoB": memoB[s],
                "i2c": np.ascontiguousarray(i2c[s]),
                "wmT": wmT,
                "wqT": wqT,
                "cw": cw,
                "wlT": wlT,
                "vT": vT,
                "qin": np.ascontiguousarray(qT[:, :, s]),
                "ident": ident,
            }
        )
    return in_maps


def kernel(query, memory, attention_weights_cat, W_query, W_memory, conv_w, W_loc, v):
    global LAST_RESULTS
    if "nc" not in _cache:
        _cache["nc"] = _build()
    nc = _cache["nc"]
    in_maps = _prep(
        query, memory, attention_weights_cat, W_query, W_memory, conv_w, W_loc, v
    )
    import os

    trace = bool(os.environ.get("BASS_TRACE"))
    res = run_bass_kernel_spmd(
        nc, in_maps, core_ids=list(range(N_CORES)), trace=trace
    )
    LAST_RESULTS = res
    ctx_full = np.concatenate([r["ctx"] for r in res.results], axis=0)
    wgt_full = np.concatenate([r["wgt"] for r in res.results], axis=0)
    return ctx_full, wgt_full
